# revision 26
# baseline (speedup 1.0000x reference)
"""Trainium2 Bass kernel for nn_AttMatch (2-graph attention + SAGEConv GNN).

Self-contained: takes the full unsharded inputs of the reference problem,
shards across 8 NeuronCores internally, runs one SPMD NEFF, and gathers the
full [8192, 8192] sigmoid adjacency output.

Sharding: the concatenated target set (2*4096 rows) is row-sharded across the
8 cores (512 rows of each graph per core).  Key/value and the attention
matrix are sharded along T; the softmax over dim 0 and alpha.T @ v are
completed with a ReduceScatter (global-chunk ordered, so every core receives
exactly its own node chunk).

SAGEConv (mean aggregation, M = row-normalized adjacency operator, built on
host from the edge index) is split algebraically so that only the
attention-output-dependent part sits on the critical path:

    h = relu( M @ X @ (Wl0+Wl1) + X @ (Wr0+Wr1) + bl     <- X-only, overlaps
              - M @ (out @ Wl1) - out_own @ Wr1 )           attention
                ^^^^^^^^^^^^^^^ distributed: each core computes its
                row-block partial W_c = M[:, own_j] @ (out_own @ Wl1) and a
                second ReduceScatter sums and re-shards them.

Node features are re-replicated with one AllGather per graph per layer.
"""

import numpy as np
import ml_dtypes

import concourse.bass as bass
import concourse.bacc as bacc
import concourse.tile as tile
import concourse.mybir as mybir
from concourse.bass_utils import run_bass_kernel_spmd

BF16 = ml_dtypes.bfloat16

N = 4096          # nodes per graph
D = 128           # feature dim (in == out == 128)
NCORES = 8
SH = N // NCORES  # 512 node shard per graph per core
ICW = 1024        # query-chunk width
NIC = N // ICW    # 4 query chunks
NT = 2 * SH // 128  # 8 local target tiles of 128 (512 of each graph)
NJ = N // 128     # 32 source-node tiles
INV_SCALE = 1.0 / np.sqrt(128.0)

F32 = mybir.dt.float32
BF = mybir.dt.bfloat16

ADD = mybir.AluOpType.add
SUB = mybir.AluOpType.subtract
MULT = mybir.AluOpType.mult
MAX = mybir.AluOpType.max

# wm indices (per layer l: base = 7*l)
WK, WQ, WV, WLS, WL1, WRS, WR1N = range(7)
IDENT = 14
# bias indices (per layer l: base = 4*l)
BK, BQ, BV, BL = range(4)

_cache = {}


def _build_nc():
    """Build and compile the SPMD Bass graph (one NeuronCore program)."""
    nc = bacc.Bacc("TRN2", target_bir_lowering=False, debug=False,
                   num_devices=NCORES)

    # ---- external I/O ----
    x1t = nc.dram_tensor("x1t", [D, N], BF, kind="ExternalInput")
    x2t = nc.dram_tensor("x2t", [D, N], BF, kind="ExternalInput")
    xgt_in = [x1t, x2t]
    xown_in = nc.dram_tensor("xown", [2, D, SH], BF, kind="ExternalInput")
    # column shard of M^T (for the own-column aggregation)
    mtc_in = [nc.dram_tensor("mtc1", [NJ, 128, SH], BF, kind="ExternalInput"),
              nc.dram_tensor("mtc2", [NJ, 128, SH], BF, kind="ExternalInput")]
    wm_in = nc.dram_tensor("wm", [15, 128, 128], BF, kind="ExternalInput")
    bs_in = nc.dram_tensor("bs", [8, 128, 1], F32, kind="ExternalInput")
    out_ext = nc.dram_tensor("out", [2, SH, 2 * N], F32, kind="ExternalOutput")

    # ---- internal DRAM for collectives ----
    rg = [list(range(NCORES))]
    ar_in = [[nc.dram_tensor(f"ar_in_{l}_{g}", [4, 129, ICW], BF)
              for g in range(2)] for l in range(2)]
    ar_out = [[nc.dram_tensor(f"ar_out_{l}_{g}", [4, 129, ICW], BF,
                              addr_space="Shared")
               for g in range(2)] for l in range(2)]
    hag_in = [[nc.dram_tensor(f"hag_in_{l}_{g}", [D, SH], BF)
               for g in range(2)] for l in range(2)]
    hag_out = [[nc.dram_tensor(f"hag_out_{l}_{g}", [NCORES, D, SH], BF,
                               addr_space="Shared")
                for g in range(2)] for l in range(2)]

    with tile.TileContext(nc) as tc:
        with (
            tc.tile_pool(name="const", bufs=1) as cpool,
            tc.tile_pool(name="xt", bufs=2) as xt_pool,
            tc.tile_pool(name="small", bufs=2) as spool,
            tc.tile_pool(name="kqv", bufs=1) as kqv_pool,
            tc.tile_pool(name="es", bufs=6) as es_pool,
            tc.tile_pool(name="csacc", bufs=2) as cs_pool,
            tc.tile_pool(name="stage", bufs=3) as st_pool,
            tc.tile_pool(name="ybig", bufs=1) as y_pool,
            tc.tile_pool(name="mt", bufs=1) as mt_pool,
            tc.tile_pool(name="z", bufs=5) as z_pool,
            tc.tile_pool(name="ps", bufs=2, space="PSUM") as ps_pool,
            tc.tile_pool(name="ps_p", bufs=3, space="PSUM") as psp_pool,
            tc.tile_pool(name="ps_cs", bufs=1, space="PSUM") as pscs_pool,
        ):
            # ---- load constants ----
            wm = cpool.tile([128, 15 * 128], BF, name="wm_sb")
            nc.scalar.dma_start(
                wm.rearrange("p (i f) -> p i f", i=15),
                wm_in.ap().rearrange("i p f -> p i f"))
            bs = cpool.tile([128, 8], F32, name="bs_sb")
            nc.scalar.dma_start(
                bs.rearrange("p (i f) -> p i f", i=8),
                bs_in.ap().rearrange("i p f -> p i f"))
            ones_m1 = cpool.tile([128, 1], BF, name="ones_m1")
            nc.vector.memset(ones_m1[:], 1.0)
            ones_row = cpool.tile([1, 128], BF, name="ones_row")
            nc.vector.memset(ones_row[:], 1.0)

            def W(l, i):
                base = 7 * l + i if i < 7 else IDENT
                return wm[:, 128 * base:128 * (base + 1)]

            def B(l, i):
                return bs[:, 4 * l + i:4 * l + i + 1]

            ident = wm[:, 128 * IDENT:128 * (IDENT + 1)]

            # ---- load inputs (generation 0) ----
            xgt = []
            for g in range(2):
                t = xt_pool.tile([D, N], BF, name=f"x{g}t_0", tag=f"xt{g}")
                nc.scalar.dma_start(t[:], xgt_in[g][:])
                xgt.append(t)
            xown = []
            for g in range(2):
                t = spool.tile([D, SH], BF, name=f"xown{g}_0", tag=f"xo{g}")
                nc.scalar.dma_start(t[:], xown_in[g])
                xown.append(t)

            hown_final = [None, None]

            for l in range(2):
                # ---- projections ----
                kt = kqv_pool.tile([D, 2 * SH], BF, name=f"kt_{l}", tag="kt",
                                   bufs=2)
                vnat = []
                for g in range(2):
                    ps = ps_pool.tile([128, 512], F32, tag="ps",
                                      name=f"psk_{l}_{g}")
                    nc.tensor.matmul(ps[:], W(l, WK), xown[g][:],
                                     start=True, stop=True)
                    nc.vector.tensor_scalar(kt[:, g * SH:(g + 1) * SH], ps[:],
                                            B(l, BK), None, ADD)
                    # v^T then transpose to natural [t, d] tiles (no bias; bv
                    # is folded in after the softmax division)
                    ps2 = ps_pool.tile([128, 512], F32, tag="ps",
                                       name=f"psv_{l}_{g}")
                    nc.tensor.matmul(ps2[:], W(l, WV), xown[g][:],
                                     start=True, stop=True)
                    vt = st_pool.tile([128, SH], BF, name=f"vt_{l}_{g}",
                                      tag="vt", bufs=2)
                    nc.vector.tensor_copy(vt[:], ps2[:])
                    psv = ps_pool.tile([128, 512], BF, tag="ps",
                                       name=f"psvt_{l}_{g}")
                    for j in range(4):
                        nc.tensor.transpose(psv[:, j * 128:(j + 1) * 128],
                                            vt[:, j * 128:(j + 1) * 128],
                                            ident)
                    vb = st_pool.tile([128, 512], BF, name=f"vn_{l}_{g}",
                                      tag=f"vn{g}", bufs=1)
                    nc.vector.tensor_copy(vb[:], psv[:])
                    vnat += [vb[:, j * 128:(j + 1) * 128] for j in range(4)]
                qt = []
                for g in range(2):
                    q = kqv_pool.tile([D, N], BF, name=f"qt_{l}_{g}",
                                      tag=f"qt{g}", bufs=1)
                    for ic in range(NIC):
                        ps = ps_pool.tile([128, ICW], F32, tag="ps",
                                          name=f"psq_{l}_{g}_{ic}")
                        for h in range(2):
                            nc.tensor.matmul(
                                ps[:, h * 512:(h + 1) * 512], W(l, WQ),
                                xgt[g][:, ic * ICW + h * 512:
                                        ic * ICW + (h + 1) * 512],
                                start=True, stop=True)
                        nc.vector.tensor_scalar(q[:, ic * ICW:(ic + 1) * ICW],
                                                ps[:], B(l, BQ), None, ADD)
                    qt.append(q)

                ls = [None, None]      # local X-only SAGE terms [d, own]
                hown = [None, None]

                def attention(g):
                    for ic in range(NIC):
                        php = [psp_pool.tile([128, 512], F32, tag="ps_p",
                                             name=f"php{h}_{l}_{g}_{ic}")
                               for h in range(2)]
                        csa = cs_pool.tile([128, ICW], BF, tag="cs")
                        for tt in range(NT):
                            ps_s = ps_pool.tile([128, ICW], F32, tag="ps")
                            for h in range(2):
                                nc.tensor.matmul(
                                    ps_s[:, h * 512:(h + 1) * 512],
                                    kt[:, tt * 128:(tt + 1) * 128],
                                    qt[g][:, ic * ICW + h * 512:
                                            ic * ICW + (h + 1) * 512],
                                    start=True, stop=True)
                            es = es_pool.tile([128, ICW], BF, tag="es")
                            nc.scalar.activation(
                                es[:], ps_s[:],
                                mybir.ActivationFunctionType.Exp,
                                scale=INV_SCALE)
                            for h in range(2):
                                nc.tensor.matmul(
                                    php[h][:], vnat[tt],
                                    es[:, h * 512:(h + 1) * 512],
                                    start=(tt == 0), stop=(tt == NT - 1))
                            if tt == 0:
                                nc.vector.tensor_copy(csa[:], es[:])
                            else:
                                nc.vector.tensor_tensor(csa[:], csa[:], es[:],
                                                        ADD)
                        pc = st_pool.tile([128, ICW], BF, tag="pc")
                        cc = st_pool.tile([1, ICW], BF, tag="cc")
                        for h in range(2):
                            nc.vector.tensor_copy(
                                pc[:, h * 512:(h + 1) * 512], php[h][:])
                            ps_c = pscs_pool.tile([1, 512], F32, tag="ps_cs")
                            nc.tensor.matmul(ps_c[:], ones_m1[:],
                                             csa[:, h * 512:(h + 1) * 512],
                                             start=True, stop=True)
                            nc.vector.tensor_copy(
                                cc[:, h * 512:(h + 1) * 512], ps_c[:])
                        nc.scalar.dma_start(
                            ar_in[l][g][ic, 0:128, :], pc[:])
                        nc.scalar.dma_start(
                            ar_in[l][g][ic, 128:129, :], cc[:])
                        nc.gpsimd.collective_compute(
                            "AllReduce", ADD, replica_groups=rg,
                            ins=[ar_in[l][g][ic]],
                            outs=[ar_out[l][g][ic]])

                def xsage(g):
                    """X-only SAGE terms: M @ X @ Wls + X_own @ Wrs (+ bl).
                    Independent of the attention output; fills PE slack."""
                    ybig = y_pool.tile([128, NJ * 128], BF,
                                       name=f"y_{l}_{g}", tag=f"y{g}")
                    for jb in range(NJ // 4):
                        psy = pscs_pool.tile([128, 512], F32, tag="ps_cs",
                                             name=f"psy_{l}_{g}_{jb}")
                        for k in range(4):
                            jt = jb * 4 + k
                            nc.tensor.matmul(psy[:, k * 128:(k + 1) * 128],
                                             xgt[g][:, jt * 128:(jt + 1) * 128],
                                             W(l, WLS), start=True, stop=True)
                        nc.vector.tensor_copy(
                            ybig[:, jb * 512:(jb + 1) * 512], psy[:])
                    ps_a = psp_pool.tile([128, 512], F32, tag="ps_p",
                                         name=f"psa_{l}_{g}")
                    for jb in range(NJ // 4):
                        mtc_t = mt_pool.tile([128, 4 * SH], BF, tag="mtcs",
                                             bufs=4,
                                             name=f"mtc_{l}_{g}_{jb}")
                        nc.scalar.dma_start(
                            mtc_t.rearrange("p (j n) -> p j n", j=4),
                            mtc_in[g].ap()[4 * jb:4 * jb + 4]
                            .rearrange("j p n -> p j n"))
                        for k in range(4):
                            jt = jb * 4 + k
                            nc.tensor.matmul(
                                ps_a[:], ybig[:, jt * 128:(jt + 1) * 128],
                                mtc_t[:, k * SH:(k + 1) * SH],
                                start=(jt == 0), stop=False)
                    nc.tensor.matmul(ps_a[:], W(l, WRS), xown[g][:],
                                     start=False, stop=True)
                    t = spool.tile([128, SH], F32, name=f"ls_{l}_{g}",
                                   tag=f"ls{g}")
                    nc.vector.tensor_scalar(t[:], ps_a[:], B(l, BL), None,
                                            ADD)
                    ls[g] = t

                def tail(g):
                    """Post-AllReduce: softmax epilogue on the full width,
                    local out-part aggregation, h for own rows, AllGather."""
                    pfull = spool.tile([128, N], BF, name=f"pfull_{l}_{g}",
                                       tag="pfull", bufs=1)
                    csrow = spool.tile([1, N], BF, name=f"csrow_{l}_{g}",
                                       tag="csrow")
                    outt = spool.tile([128, N], BF, name=f"outt_{l}_{g}",
                                      tag="outt", bufs=1)
                    for hh in range(4):
                        nc.scalar.dma_start(
                            pfull[:, hh * ICW:(hh + 1) * ICW],
                            ar_out[l][g][hh, 0:128, :])
                        nc.scalar.dma_start(
                            csrow[:, hh * ICW:(hh + 1) * ICW],
                            ar_out[l][g][hh, 128:129, :])
                        for chq in range(2):
                            ch = hh * 2 + chq
                            sl = slice(ch * 512, (ch + 1) * 512)
                            ps_rep = psp_pool.tile([128, 512], F32,
                                                   tag="ps_p",
                                                   name=f"psrep_{l}_{g}_{ch}")
                            nc.tensor.matmul(ps_rep[:], ones_row[:],
                                             csrow[:, sl],
                                             start=True, stop=True)
                            rr = spool.tile([128, 512], F32,
                                            name=f"rr_{l}_{g}", tag="rr")
                            nc.vector.reciprocal_approx_fast(rr[:], ps_rep[:])
                            prod = spool.tile([128, 512], BF,
                                              name=f"prod_{l}_{g}",
                                              tag="prod")
                            nc.vector.tensor_tensor(prod[:], pfull[:, sl],
                                                    rr[:], MULT)
                            nc.vector.tensor_scalar(outt[:, sl], prod[:],
                                                    B(l, BV), None, ADD)
                    # own columns of out (runtime core offset)
                    oown = spool.tile([128, 512], BF, name=f"oown_{l}_{g}",
                                      tag="oown")
                    pid = nc.gpsimd.partition_id()
                    nc.gpsimd.dma_start(oown[:],
                                        outt[:, bass.ds(pid * SH, SH)])
                    # U = out @ Wl1  (natural [j, d] tiles, all 4096 j)
                    ubig = spool.tile([128, N], BF, name=f"ubig_{l}_{g}",
                                      tag="ubig", bufs=1)
                    for jb in range(8):
                        ps_u = pscs_pool.tile([128, 512], F32, tag="ps_cs",
                                              name=f"psu_{l}_{g}_{jb}")
                        for k in range(4):
                            jt = jb * 4 + k
                            nc.tensor.matmul(
                                ps_u[:, k * 128:(k + 1) * 128],
                                outt[:, jt * 128:(jt + 1) * 128],
                                W(l, WL1), start=True, stop=True)
                        nc.vector.tensor_copy(
                            ubig[:, jb * 512:(jb + 1) * 512], ps_u[:])
                    # P2 = M @ U |own + out_own @ Wr1   (to subtract from ls)
                    ps_a2 = psp_pool.tile([128, 512], F32, tag="ps_p",
                                          name=f"psa2_{l}_{g}")
                    for jb in range(NJ // 4):
                        mtc_t = mt_pool.tile([128, 4 * SH], BF, tag="mtcs",
                                             bufs=4,
                                             name=f"mtc2_{l}_{g}_{jb}")
                        nc.scalar.dma_start(
                            mtc_t.rearrange("p (j n) -> p j n", j=4),
                            mtc_in[g].ap()[4 * jb:4 * jb + 4]
                            .rearrange("j p n -> p j n"))
                        for k in range(4):
                            jt = jb * 4 + k
                            nc.tensor.matmul(
                                ps_a2[:], ubig[:, jt * 128:(jt + 1) * 128],
                                mtc_t[:, k * SH:(k + 1) * SH],
                                start=(jt == 0), stop=False)
                    nc.tensor.matmul(ps_a2[:], W(l, WR1N), oown[:],
                                     start=False, stop=True)
                    h = spool.tile([D, SH], BF, name=f"hown_{l}_{g}",
                                   tag=f"xo{g}")
                    if l == 0:
                        t2 = spool.tile([128, 512], F32, name=f"t2_{l}_{g}",
                                        tag="t2")
                        nc.vector.tensor_tensor(t2[:], ls[g][:], ps_a2[:],
                                                SUB)
                        nc.vector.tensor_scalar(h[:], t2[:], 0.0, None, MAX)
                    else:
                        nc.vector.tensor_tensor(h[:], ls[g][:], ps_a2[:],
                                                SUB)
                    hown[g] = h
                    nc.scalar.dma_start(hag_in[l][g][:], h[:])
                    nc.gpsimd.collective_compute(
                        "AllGather", mybir.AluOpType.bypass, replica_groups=rg,
                        ins=[hag_in[l][g][:]], outs=[hag_out[l][g][:]])

                xsage(0)
                xsage(1)
                attention(0)
                tail(0)
                attention(1)
                tail(1)

                # ---- gather new X generation ----
                new_xgt = []
                for g in range(2):
                    t = xt_pool.tile([D, N], BF, name=f"x{g}t_{l + 1}",
                                     tag=f"xt{g}")
                    nc.scalar.dma_start(
                        t.rearrange("p (c n) -> p c n", c=NCORES),
                        hag_out[l][g].ap().rearrange("c p n -> p c n"))
                    new_xgt.append(t)
                xgt = new_xgt
                xown = hown
                if l == 1:
                    hown_final = hown

            # ---- final adjacency: sigmoid(F @ F^T), own 1024 rows ----
            for g in range(2):
                for rt in range(4):
                    lhs = hown_final[g][:, rt * 128:(rt + 1) * 128]
                    for cb in range(8):
                        src = xgt[cb // 4]
                        c0 = (cb % 4) * ICW
                        ps_z = ps_pool.tile([128, ICW], F32, tag="ps")
                        for h in range(2):
                            nc.tensor.matmul(
                                ps_z[:, h * 512:(h + 1) * 512], lhs,
                                src[:, c0 + h * 512:c0 + (h + 1) * 512],
                                start=True, stop=True)
                        z = z_pool.tile([128, ICW], F32, tag="z")
                        nc.scalar.activation(
                            z[:], ps_z[:],
                            mybir.ActivationFunctionType.Sigmoid)
                        nc.scalar.dma_start(
                            out_ext[g, rt * 128:(rt + 1) * 128,
                                    cb * ICW:cb * ICW + 512],
                            z[:, 0:512])
                        nc.gpsimd.dma_start(
                            out_ext[g, rt * 128:(rt + 1) * 128,
                                    cb * ICW + 512:(cb + 1) * ICW],
                            z[:, 512:1024])

    nc.compile()
    return nc


def _host_prep(inputs):
    """Build per-core input maps from the full problem inputs."""
    x1 = np.asarray(inputs["x1"], np.float32)
    x2 = np.asarray(inputs["x2"], np.float32)
    x1t = np.ascontiguousarray(x1.T).astype(BF16)
    x2t = np.ascontiguousarray(x2.T).astype(BF16)

    def norm_adj_t(ei):
        ei = np.asarray(ei)
        A = np.zeros((N, N), np.float32)
        np.add.at(A, (ei[1], ei[0]), 1.0)
        deg = A.sum(1)
        A /= np.maximum(deg, 1.0)[:, None]
        return np.ascontiguousarray(A.T)  # MT[j, n]

    mt = [norm_adj_t(inputs["ei1"]), norm_adj_t(inputs["ei2"])]

    wm = np.zeros((15, 128, 128), np.float32)
    bs = np.zeros((8, 128, 1), np.float32)
    for l, s in enumerate(("1", "2")):
        wm[7 * l + WK] = inputs["Wk" + s]
        wm[7 * l + WQ] = inputs["Wq" + s]
        wm[7 * l + WV] = inputs["Wv" + s]
        wm[7 * l + WLS] = inputs["Wl" + s][:128] + inputs["Wl" + s][128:]
        wm[7 * l + WL1] = inputs["Wl" + s][128:]
        wm[7 * l + WRS] = inputs["Wr" + s][:128] + inputs["Wr" + s][128:]
        wm[7 * l + WR1N] = inputs["Wr" + s][128:]
        bs[4 * l + BK, :, 0] = inputs["bk" + s]
        bs[4 * l + BQ, :, 0] = inputs["bq" + s]
        bs[4 * l + BV, :, 0] = inputs["bv" + s]
        bs[4 * l + BL, :, 0] = inputs["bl" + s]
    wm[IDENT] = np.eye(128)
    wm = wm.astype(BF16)

    in_maps = []
    for c in range(NCORES):
        sl = slice(c * SH, (c + 1) * SH)
        in_maps.append({
            "x1t": x1t,
            "x2t": x2t,
            "xown": np.stack([x1t[:, sl], x2t[:, sl]]),
            "mtc1": np.ascontiguousarray(
                mt[0][:, sl].astype(BF16).reshape(NJ, 128, SH)),
            "mtc2": np.ascontiguousarray(
                mt[1][:, sl].astype(BF16).reshape(NJ, 128, SH)),
            "wm": wm,
            "bs": bs,
        })
    return in_maps


def _assemble(results):
    full = np.empty((2 * N, 2 * N), np.float32)
    for c in range(NCORES):
        o = results[c]["out"]
        full[c * SH:(c + 1) * SH] = o[0]
        full[N + c * SH:N + (c + 1) * SH] = o[1]
    return full


def get_nc():
    if "nc" not in _cache:
        _cache["nc"] = _build_nc()
    return _cache["nc"]


def kernel(**inputs):
    nc = get_nc()
    in_maps = _host_prep(inputs)
    res = run_bass_kernel_spmd(nc, in_maps, core_ids=list(range(NCORES)))
    return _assemble(res.results)


# revision 28
# speedup vs baseline: 1.0958x; 1.0958x over previous
"""Trainium2 Bass kernel for nn_AttMatch (2-graph attention + SAGEConv GNN).

Self-contained: takes the full unsharded inputs of the reference problem,
shards across 8 NeuronCores internally, runs one SPMD NEFF, and gathers the
full [8192, 8192] sigmoid adjacency output.

Sharding: the concatenated target set (2*4096 rows) is row-sharded across the
8 cores (512 rows of each graph per core).  Key/value and the attention
matrix are sharded along T; the softmax over dim 0 and alpha.T @ v are
completed with a ReduceScatter (global-chunk ordered, so every core receives
exactly its own node chunk).

SAGEConv (mean aggregation, M = row-normalized adjacency operator, built on
host from the edge index) is split algebraically so that only the
attention-output-dependent part sits on the critical path:

    h = relu( M @ X @ (Wl0+Wl1) + X @ (Wr0+Wr1) + bl     <- X-only, overlaps
              - M @ (out @ Wl1) - out_own @ Wr1 )           attention
                ^^^^^^^^^^^^^^^ distributed: each core computes its
                row-block partial W_c = M[:, own_j] @ (out_own @ Wl1) and a
                second ReduceScatter sums and re-shards them.

Node features are re-replicated with one AllGather per graph per layer.
"""

import numpy as np
import ml_dtypes

import concourse.bass as bass
import concourse.bacc as bacc
import concourse.tile as tile
import concourse.mybir as mybir
from concourse.bass_utils import run_bass_kernel_spmd

BF16 = ml_dtypes.bfloat16

N = 4096          # nodes per graph
D = 128           # feature dim (in == out == 128)
NCORES = 8
SH = N // NCORES  # 512 node shard per graph per core
ICW = 1024        # query-chunk width
NIC = N // ICW    # 4 query chunks
NT = 2 * SH // 128  # 8 local target tiles of 128 (512 of each graph)
NJ = N // 128     # 32 source-node tiles
INV_SCALE = 1.0 / np.sqrt(128.0)

F32 = mybir.dt.float32
BF = mybir.dt.bfloat16

ADD = mybir.AluOpType.add
SUB = mybir.AluOpType.subtract
MULT = mybir.AluOpType.mult
MAX = mybir.AluOpType.max

# wm indices (per layer l: base = 7*l)
WK, WQ, WV, WLS, WL1, WRS, WR1N = range(7)
IDENT = 14
# bias indices (per layer l: base = 4*l)
BK, BQ, BV, BL = range(4)

_cache = {}


def _build_nc():
    """Build and compile the SPMD Bass graph (one NeuronCore program)."""
    nc = bacc.Bacc("TRN2", target_bir_lowering=False, debug=False,
                   num_devices=NCORES)

    # ---- external I/O ----
    x1t = nc.dram_tensor("x1t", [D, N], BF, kind="ExternalInput")
    x2t = nc.dram_tensor("x2t", [D, N], BF, kind="ExternalInput")
    xgt_in = [x1t, x2t]
    xown_in = nc.dram_tensor("xown", [2, D, SH], BF, kind="ExternalInput")
    # column shard of M^T (for the own-column aggregation)
    mtc_in = [nc.dram_tensor("mtc1", [NJ, 128, SH], BF, kind="ExternalInput"),
              nc.dram_tensor("mtc2", [NJ, 128, SH], BF, kind="ExternalInput")]
    wm_in = nc.dram_tensor("wm", [15, 128, 128], BF, kind="ExternalInput")
    bs_in = nc.dram_tensor("bs", [8, 128, 1], F32, kind="ExternalInput")
    out_ext = nc.dram_tensor("out", [2, SH, 2 * N], F32, kind="ExternalOutput")

    # ---- internal DRAM for collectives ----
    rg = [list(range(NCORES))]
    ar_in = [[nc.dram_tensor(f"ar_in_{l}_{g}", [2, 129, N // 2], BF)
              for g in range(2)] for l in range(2)]
    ar_out = [[nc.dram_tensor(f"ar_out_{l}_{g}", [2, 129, N // 2], BF,
                              addr_space="Shared")
               for g in range(2)] for l in range(2)]
    hag_in = [[nc.dram_tensor(f"hag_in_{l}_{g}", [D, SH], BF)
               for g in range(2)] for l in range(2)]
    hag_out = [[nc.dram_tensor(f"hag_out_{l}_{g}", [NCORES, D, SH], BF,
                               addr_space="Shared")
                for g in range(2)] for l in range(2)]

    with tile.TileContext(nc) as tc:
        with (
            tc.tile_pool(name="const", bufs=1) as cpool,
            tc.tile_pool(name="xt", bufs=2) as xt_pool,
            tc.tile_pool(name="small", bufs=2) as spool,
            tc.tile_pool(name="kqv", bufs=1) as kqv_pool,
            tc.tile_pool(name="es", bufs=6) as es_pool,
            tc.tile_pool(name="csacc", bufs=2) as cs_pool,
            tc.tile_pool(name="stage", bufs=3) as st_pool,
            tc.tile_pool(name="ybig", bufs=1) as y_pool,
            tc.tile_pool(name="mt", bufs=1) as mt_pool,
            tc.tile_pool(name="z", bufs=5) as z_pool,
            tc.tile_pool(name="ps", bufs=2, space="PSUM") as ps_pool,
            tc.tile_pool(name="ps_p", bufs=3, space="PSUM") as psp_pool,
            tc.tile_pool(name="ps_cs", bufs=1, space="PSUM") as pscs_pool,
        ):
            # ---- load constants ----
            wm = cpool.tile([128, 15 * 128], BF, name="wm_sb")
            nc.scalar.dma_start(
                wm.rearrange("p (i f) -> p i f", i=15),
                wm_in.ap().rearrange("i p f -> p i f"))
            bs = cpool.tile([128, 8], F32, name="bs_sb")
            nc.scalar.dma_start(
                bs.rearrange("p (i f) -> p i f", i=8),
                bs_in.ap().rearrange("i p f -> p i f"))
            ones_m1 = cpool.tile([128, 1], BF, name="ones_m1")
            nc.vector.memset(ones_m1[:], 1.0)
            ones_row = cpool.tile([1, 128], BF, name="ones_row")
            nc.vector.memset(ones_row[:], 1.0)

            def W(l, i):
                base = 7 * l + i if i < 7 else IDENT
                return wm[:, 128 * base:128 * (base + 1)]

            def B(l, i):
                return bs[:, 4 * l + i:4 * l + i + 1]

            ident = wm[:, 128 * IDENT:128 * (IDENT + 1)]

            # ---- load inputs (generation 0) ----
            xgt = []
            for g in range(2):
                t = xt_pool.tile([D, N], BF, name=f"x{g}t_0", tag=f"xt{g}")
                nc.scalar.dma_start(t[:], xgt_in[g][:])
                xgt.append(t)
            xown = []
            for g in range(2):
                t = spool.tile([D, SH], BF, name=f"xown{g}_0", tag=f"xo{g}")
                nc.scalar.dma_start(t[:], xown_in[g])
                xown.append(t)

            hown_final = [None, None]

            for l in range(2):
                # ---- projections ----
                kt = kqv_pool.tile([D, 2 * SH], BF, name=f"kt_{l}", tag="kt",
                                   bufs=2)
                vnat = []
                for g in range(2):
                    ps = ps_pool.tile([128, 512], F32, tag="ps",
                                      name=f"psk_{l}_{g}")
                    nc.tensor.matmul(ps[:], W(l, WK), xown[g][:],
                                     start=True, stop=True)
                    nc.vector.tensor_scalar(kt[:, g * SH:(g + 1) * SH], ps[:],
                                            B(l, BK), None, ADD)
                    # v^T then transpose to natural [t, d] tiles (no bias; bv
                    # is folded in after the softmax division)
                    ps2 = ps_pool.tile([128, 512], F32, tag="ps",
                                       name=f"psv_{l}_{g}")
                    nc.tensor.matmul(ps2[:], W(l, WV), xown[g][:],
                                     start=True, stop=True)
                    vt = st_pool.tile([128, SH], BF, name=f"vt_{l}_{g}",
                                      tag="vt", bufs=2)
                    nc.vector.tensor_copy(vt[:], ps2[:])
                    psv = ps_pool.tile([128, 512], BF, tag="ps",
                                       name=f"psvt_{l}_{g}")
                    for j in range(4):
                        nc.tensor.transpose(psv[:, j * 128:(j + 1) * 128],
                                            vt[:, j * 128:(j + 1) * 128],
                                            ident)
                    vb = st_pool.tile([128, 512], BF, name=f"vn_{l}_{g}",
                                      tag=f"vn{g}", bufs=1)
                    nc.vector.tensor_copy(vb[:], psv[:])
                    vnat += [vb[:, j * 128:(j + 1) * 128] for j in range(4)]
                qt = []
                for g in range(2):
                    q = kqv_pool.tile([D, N], BF, name=f"qt_{l}_{g}",
                                      tag=f"qt{g}", bufs=1)
                    for ic in range(NIC):
                        ps = ps_pool.tile([128, ICW], F32, tag="ps",
                                          name=f"psq_{l}_{g}_{ic}")
                        for h in range(2):
                            nc.tensor.matmul(
                                ps[:, h * 512:(h + 1) * 512], W(l, WQ),
                                xgt[g][:, ic * ICW + h * 512:
                                        ic * ICW + (h + 1) * 512],
                                start=True, stop=True)
                        nc.vector.tensor_scalar(q[:, ic * ICW:(ic + 1) * ICW],
                                                ps[:], B(l, BQ), None, ADD)
                    qt.append(q)

                ls = [None, None]      # local X-only SAGE terms [d, own]
                hown = [None, None]

                def attention(g):
                    for ic in range(NIC):
                        php = [psp_pool.tile([128, 512], F32, tag="ps_p",
                                             name=f"php{h}_{l}_{g}_{ic}")
                               for h in range(2)]
                        csa = cs_pool.tile([128, ICW], BF, tag="cs")
                        for tt in range(NT):
                            ps_s = ps_pool.tile([128, ICW], F32, tag="ps")
                            for h in range(2):
                                nc.tensor.matmul(
                                    ps_s[:, h * 512:(h + 1) * 512],
                                    kt[:, tt * 128:(tt + 1) * 128],
                                    qt[g][:, ic * ICW + h * 512:
                                            ic * ICW + (h + 1) * 512],
                                    start=True, stop=True)
                            es = es_pool.tile([128, ICW], BF, tag="es")
                            nc.scalar.activation(
                                es[:], ps_s[:],
                                mybir.ActivationFunctionType.Exp,
                                scale=INV_SCALE)
                            for h in range(2):
                                nc.tensor.matmul(
                                    php[h][:], vnat[tt],
                                    es[:, h * 512:(h + 1) * 512],
                                    start=(tt == 0), stop=(tt == NT - 1))
                            if tt == 0:
                                nc.vector.tensor_copy(csa[:], es[:])
                            else:
                                nc.vector.tensor_tensor(csa[:], csa[:], es[:],
                                                        ADD)
                        pc = st_pool.tile([128, ICW], BF, tag="pc")
                        cc = st_pool.tile([1, ICW], BF, tag="cc")
                        for h in range(2):
                            nc.vector.tensor_copy(
                                pc[:, h * 512:(h + 1) * 512], php[h][:])
                            ps_c = pscs_pool.tile([1, 512], F32, tag="ps_cs")
                            nc.tensor.matmul(ps_c[:], ones_m1[:],
                                             csa[:, h * 512:(h + 1) * 512],
                                             start=True, stop=True)
                            nc.vector.tensor_copy(
                                cc[:, h * 512:(h + 1) * 512], ps_c[:])
                        hh, icq = divmod(ic, 2)
                        nc.scalar.dma_start(
                            ar_in[l][g][hh, 0:128,
                                        icq * ICW:(icq + 1) * ICW],
                            pc[:])
                        nc.scalar.dma_start(
                            ar_in[l][g][hh, 128:129,
                                        icq * ICW:(icq + 1) * ICW],
                            cc[:])
                        if ic % 2 == 1:
                            nc.gpsimd.collective_compute(
                                "AllReduce", ADD, replica_groups=rg,
                                ins=[ar_in[l][g][hh]],
                                outs=[ar_out[l][g][hh]])

                def xsage(g):
                    """X-only SAGE terms: M @ X @ Wls + X_own @ Wrs (+ bl).
                    Independent of the attention output; fills PE slack."""
                    ybig = y_pool.tile([128, NJ * 128], BF,
                                       name=f"y_{l}_{g}", tag=f"y{g}")
                    for jb in range(NJ // 4):
                        psy = pscs_pool.tile([128, 512], F32, tag="ps_cs",
                                             name=f"psy_{l}_{g}_{jb}")
                        for k in range(4):
                            jt = jb * 4 + k
                            nc.tensor.matmul(psy[:, k * 128:(k + 1) * 128],
                                             xgt[g][:, jt * 128:(jt + 1) * 128],
                                             W(l, WLS), start=True, stop=True)
                        nc.vector.tensor_copy(
                            ybig[:, jb * 512:(jb + 1) * 512], psy[:])
                    ps_a = psp_pool.tile([128, 512], F32, tag="ps_p",
                                         name=f"psa_{l}_{g}")
                    for jb in range(NJ // 4):
                        mtc_t = mt_pool.tile([128, 4 * SH], BF, tag="mtcs",
                                             bufs=4,
                                             name=f"mtc_{l}_{g}_{jb}")
                        nc.scalar.dma_start(
                            mtc_t.rearrange("p (j n) -> p j n", j=4),
                            mtc_in[g].ap()[4 * jb:4 * jb + 4]
                            .rearrange("j p n -> p j n"))
                        for k in range(4):
                            jt = jb * 4 + k
                            nc.tensor.matmul(
                                ps_a[:], ybig[:, jt * 128:(jt + 1) * 128],
                                mtc_t[:, k * SH:(k + 1) * SH],
                                start=(jt == 0), stop=False)
                    nc.tensor.matmul(ps_a[:], W(l, WRS), xown[g][:],
                                     start=False, stop=True)
                    t = spool.tile([128, SH], F32, name=f"ls_{l}_{g}",
                                   tag=f"ls{g}")
                    nc.vector.tensor_scalar(t[:], ps_a[:], B(l, BL), None,
                                            ADD)
                    ls[g] = t

                def tail(g):
                    """Post-AllReduce: softmax epilogue on the full width,
                    local out-part aggregation, h for own rows, AllGather."""
                    pfull = spool.tile([128, N], BF, name=f"pfull_{l}_{g}",
                                       tag="pfull", bufs=1)
                    csrow = spool.tile([1, N], BF, name=f"csrow_{l}_{g}",
                                       tag="csrow")
                    outt = spool.tile([128, N], BF, name=f"outt_{l}_{g}",
                                      tag="outt", bufs=1)
                    for hh in range(2):
                        nc.scalar.dma_start(
                            pfull[:, hh * 2048:(hh + 1) * 2048],
                            ar_out[l][g][hh, 0:128, :])
                        nc.scalar.dma_start(
                            csrow[:, hh * 2048:(hh + 1) * 2048],
                            ar_out[l][g][hh, 128:129, :])
                        for chq in range(4):
                            ch = hh * 4 + chq
                            sl = slice(ch * 512, (ch + 1) * 512)
                            ps_rep = psp_pool.tile([128, 512], F32,
                                                   tag="ps_p",
                                                   name=f"psrep_{l}_{g}_{ch}")
                            nc.tensor.matmul(ps_rep[:], ones_row[:],
                                             csrow[:, sl],
                                             start=True, stop=True)
                            rr = spool.tile([128, 512], F32,
                                            name=f"rr_{l}_{g}", tag="rr")
                            nc.vector.reciprocal_approx_fast(rr[:], ps_rep[:])
                            prod = spool.tile([128, 512], BF,
                                              name=f"prod_{l}_{g}",
                                              tag="prod")
                            nc.vector.tensor_tensor(prod[:], pfull[:, sl],
                                                    rr[:], MULT)
                            nc.vector.tensor_scalar(outt[:, sl], prod[:],
                                                    B(l, BV), None, ADD)
                    # own columns of out (runtime core offset)
                    oown = spool.tile([128, 512], BF, name=f"oown_{l}_{g}",
                                      tag="oown")
                    pid = nc.gpsimd.partition_id()
                    nc.gpsimd.dma_start(oown[:],
                                        outt[:, bass.ds(pid * SH, SH)])
                    # U = out @ Wl1  (natural [j, d] tiles, all 4096 j)
                    ubig = spool.tile([128, N], BF, name=f"ubig_{l}_{g}",
                                      tag="ubig", bufs=1)
                    for jb in range(8):
                        ps_u = pscs_pool.tile([128, 512], F32, tag="ps_cs",
                                              name=f"psu_{l}_{g}_{jb}")
                        for k in range(4):
                            jt = jb * 4 + k
                            nc.tensor.matmul(
                                ps_u[:, k * 128:(k + 1) * 128],
                                outt[:, jt * 128:(jt + 1) * 128],
                                W(l, WL1), start=True, stop=True)
                        nc.vector.tensor_copy(
                            ubig[:, jb * 512:(jb + 1) * 512], ps_u[:])
                    # P2 = M @ U |own + out_own @ Wr1   (to subtract from ls)
                    ps_a2 = psp_pool.tile([128, 512], F32, tag="ps_p",
                                          name=f"psa2_{l}_{g}")
                    for jb in range(NJ // 4):
                        mtc_t = mt_pool.tile([128, 4 * SH], BF, tag="mtcs",
                                             bufs=4,
                                             name=f"mtc2_{l}_{g}_{jb}")
                        nc.scalar.dma_start(
                            mtc_t.rearrange("p (j n) -> p j n", j=4),
                            mtc_in[g].ap()[4 * jb:4 * jb + 4]
                            .rearrange("j p n -> p j n"))
                        for k in range(4):
                            jt = jb * 4 + k
                            nc.tensor.matmul(
                                ps_a2[:], ubig[:, jt * 128:(jt + 1) * 128],
                                mtc_t[:, k * SH:(k + 1) * SH],
                                start=(jt == 0), stop=False)
                    nc.tensor.matmul(ps_a2[:], W(l, WR1N), oown[:],
                                     start=False, stop=True)
                    h = spool.tile([D, SH], BF, name=f"hown_{l}_{g}",
                                   tag=f"xo{g}")
                    if l == 0:
                        t2 = spool.tile([128, 512], F32, name=f"t2_{l}_{g}",
                                        tag="t2")
                        nc.vector.tensor_tensor(t2[:], ls[g][:], ps_a2[:],
                                                SUB)
                        nc.vector.tensor_scalar(h[:], t2[:], 0.0, None, MAX)
                    else:
                        nc.vector.tensor_tensor(h[:], ls[g][:], ps_a2[:],
                                                SUB)
                    hown[g] = h
                    nc.scalar.dma_start(hag_in[l][g][:], h[:])
                    nc.gpsimd.collective_compute(
                        "AllGather", mybir.AluOpType.bypass, replica_groups=rg,
                        ins=[hag_in[l][g][:]], outs=[hag_out[l][g][:]])

                xsage(0)
                xsage(1)
                attention(0)
                tail(0)
                attention(1)
                tail(1)

                # ---- gather new X generation ----
                new_xgt = []
                for g in range(2):
                    t = xt_pool.tile([D, N], BF, name=f"x{g}t_{l + 1}",
                                     tag=f"xt{g}")
                    nc.scalar.dma_start(
                        t.rearrange("p (c n) -> p c n", c=NCORES),
                        hag_out[l][g].ap().rearrange("c p n -> p c n"))
                    new_xgt.append(t)
                xgt = new_xgt
                xown = hown
                if l == 1:
                    hown_final = hown

            # ---- final adjacency: sigmoid(F @ F^T), own 1024 rows ----
            for g in range(2):
                for rt in range(4):
                    lhs = hown_final[g][:, rt * 128:(rt + 1) * 128]
                    for cb in range(8):
                        src = xgt[cb // 4]
                        c0 = (cb % 4) * ICW
                        z = z_pool.tile([128, ICW], F32, tag="z")
                        if cb < 4 or (rt * 8 + cb) % 2 == 0:
                            ps_z = ps_pool.tile([128, ICW], F32, tag="ps")
                            for h in range(2):
                                nc.tensor.matmul(
                                    ps_z[:, h * 512:(h + 1) * 512], lhs,
                                    src[:, c0 + h * 512:c0 + (h + 1) * 512],
                                    start=True, stop=True)
                            nc.scalar.activation(
                                z[:], ps_z[:],
                                mybir.ActivationFunctionType.Sigmoid)
                        else:
                            # x2-column chunks run strictly after the last
                            # AllGather, when the tails' ps_p banks are idle:
                            # alternate onto them to keep the PE dense
                            for h in range(2):
                                psz = psp_pool.tile(
                                    [128, 512], F32, tag="ps_p",
                                    name=f"psf_{g}_{rt}_{cb}_{h}")
                                nc.tensor.matmul(
                                    psz[:], lhs,
                                    src[:, c0 + h * 512:c0 + (h + 1) * 512],
                                    start=True, stop=True)
                                nc.scalar.activation(
                                    z[:, h * 512:(h + 1) * 512], psz[:],
                                    mybir.ActivationFunctionType.Sigmoid)
                        nc.scalar.dma_start(
                            out_ext[g, rt * 128:(rt + 1) * 128,
                                    cb * ICW:cb * ICW + 512],
                            z[:, 0:512])
                        nc.gpsimd.dma_start(
                            out_ext[g, rt * 128:(rt + 1) * 128,
                                    cb * ICW + 512:(cb + 1) * ICW],
                            z[:, 512:1024])

    nc.compile()
    return nc


def _host_prep(inputs):
    """Build per-core input maps from the full problem inputs."""
    x1 = np.asarray(inputs["x1"], np.float32)
    x2 = np.asarray(inputs["x2"], np.float32)
    x1t = np.ascontiguousarray(x1.T).astype(BF16)
    x2t = np.ascontiguousarray(x2.T).astype(BF16)

    def norm_adj_t(ei):
        ei = np.asarray(ei)
        A = np.zeros((N, N), np.float32)
        np.add.at(A, (ei[1], ei[0]), 1.0)
        deg = A.sum(1)
        A /= np.maximum(deg, 1.0)[:, None]
        return np.ascontiguousarray(A.T)  # MT[j, n]

    mt = [norm_adj_t(inputs["ei1"]), norm_adj_t(inputs["ei2"])]

    wm = np.zeros((15, 128, 128), np.float32)
    bs = np.zeros((8, 128, 1), np.float32)
    for l, s in enumerate(("1", "2")):
        wm[7 * l + WK] = inputs["Wk" + s]
        wm[7 * l + WQ] = inputs["Wq" + s]
        wm[7 * l + WV] = inputs["Wv" + s]
        wm[7 * l + WLS] = inputs["Wl" + s][:128] + inputs["Wl" + s][128:]
        wm[7 * l + WL1] = inputs["Wl" + s][128:]
        wm[7 * l + WRS] = inputs["Wr" + s][:128] + inputs["Wr" + s][128:]
        wm[7 * l + WR1N] = inputs["Wr" + s][128:]
        bs[4 * l + BK, :, 0] = inputs["bk" + s]
        bs[4 * l + BQ, :, 0] = inputs["bq" + s]
        bs[4 * l + BV, :, 0] = inputs["bv" + s]
        bs[4 * l + BL, :, 0] = inputs["bl" + s]
    wm[IDENT] = np.eye(128)
    wm = wm.astype(BF16)

    in_maps = []
    for c in range(NCORES):
        sl = slice(c * SH, (c + 1) * SH)
        in_maps.append({
            "x1t": x1t,
            "x2t": x2t,
            "xown": np.stack([x1t[:, sl], x2t[:, sl]]),
            "mtc1": np.ascontiguousarray(
                mt[0][:, sl].astype(BF16).reshape(NJ, 128, SH)),
            "mtc2": np.ascontiguousarray(
                mt[1][:, sl].astype(BF16).reshape(NJ, 128, SH)),
            "wm": wm,
            "bs": bs,
        })
    return in_maps


def _assemble(results):
    full = np.empty((2 * N, 2 * N), np.float32)
    for c in range(NCORES):
        o = results[c]["out"]
        full[c * SH:(c + 1) * SH] = o[0]
        full[N + c * SH:N + (c + 1) * SH] = o[1]
    return full


def get_nc():
    if "nc" not in _cache:
        _cache["nc"] = _build_nc()
    return _cache["nc"]


def kernel(**inputs):
    nc = get_nc()
    in_maps = _host_prep(inputs)
    res = run_bass_kernel_spmd(nc, in_maps, core_ids=list(range(NCORES)))
    return _assemble(res.results)


# revision 29
# speedup vs baseline: 1.1181x; 1.0204x over previous
"""Trainium2 Bass kernel for nn_AttMatch (2-graph attention + SAGEConv GNN).

Self-contained: takes the full unsharded inputs of the reference problem,
shards across 8 NeuronCores internally, runs one SPMD NEFF, and gathers the
full [8192, 8192] sigmoid adjacency output.

Sharding: the concatenated target set (2*4096 rows) is row-sharded across the
8 cores (512 rows of each graph per core).  Key/value and the attention
matrix are sharded along T; the softmax over dim 0 and alpha.T @ v are
completed with a ReduceScatter (global-chunk ordered, so every core receives
exactly its own node chunk).

SAGEConv (mean aggregation, M = row-normalized adjacency operator, built on
host from the edge index) is split algebraically so that only the
attention-output-dependent part sits on the critical path:

    h = relu( M @ X @ (Wl0+Wl1) + X @ (Wr0+Wr1) + bl     <- X-only, overlaps
              - M @ (out @ Wl1) - out_own @ Wr1 )           attention
                ^^^^^^^^^^^^^^^ distributed: each core computes its
                row-block partial W_c = M[:, own_j] @ (out_own @ Wl1) and a
                second ReduceScatter sums and re-shards them.

Node features are re-replicated with one AllGather per graph per layer.
"""

import numpy as np
import ml_dtypes

import concourse.bass as bass
import concourse.bacc as bacc
import concourse.tile as tile
import concourse.mybir as mybir
from concourse.bass_utils import run_bass_kernel_spmd

BF16 = ml_dtypes.bfloat16

N = 4096          # nodes per graph
D = 128           # feature dim (in == out == 128)
NCORES = 8
SH = N // NCORES  # 512 node shard per graph per core
ICW = 1024        # query-chunk width
NIC = N // ICW    # 4 query chunks
NT = 2 * SH // 128  # 8 local target tiles of 128 (512 of each graph)
NJ = N // 128     # 32 source-node tiles
INV_SCALE = 1.0 / np.sqrt(128.0)

F32 = mybir.dt.float32
BF = mybir.dt.bfloat16

ADD = mybir.AluOpType.add
SUB = mybir.AluOpType.subtract
MULT = mybir.AluOpType.mult
MAX = mybir.AluOpType.max

# wm indices (per layer l: base = 7*l)
WK, WQ, WV, WLS, WL1, WRS, WR1N = range(7)
IDENT = 14
# bias indices (per layer l: base = 4*l)
BK, BQ, BV, BL = range(4)

_cache = {}


def _build_nc():
    """Build and compile the SPMD Bass graph (one NeuronCore program)."""
    nc = bacc.Bacc("TRN2", target_bir_lowering=False, debug=False,
                   num_devices=NCORES)

    # ---- external I/O ----
    x1t = nc.dram_tensor("x1t", [D, N], BF, kind="ExternalInput")
    x2t = nc.dram_tensor("x2t", [D, N], BF, kind="ExternalInput")
    xgt_in = [x1t, x2t]
    xown_in = nc.dram_tensor("xown", [2, D, SH], BF, kind="ExternalInput")
    # column shard of M^T (for the own-column aggregation)
    mtc_in = [nc.dram_tensor("mtc1", [NJ, 128, SH], BF, kind="ExternalInput"),
              nc.dram_tensor("mtc2", [NJ, 128, SH], BF, kind="ExternalInput")]
    wm_in = nc.dram_tensor("wm", [15, 128, 128], BF, kind="ExternalInput")
    bs_in = nc.dram_tensor("bs", [8, 128, 1], F32, kind="ExternalInput")
    out_ext = nc.dram_tensor("out", [2, SH, 2 * N], F32, kind="ExternalOutput")

    # ---- internal DRAM for collectives ----
    rg = [list(range(NCORES))]
    ar_in = [[nc.dram_tensor(f"ar_in_{l}_{g}", [2, 129, N // 2], BF)
              for g in range(2)] for l in range(2)]
    ar_out = [[nc.dram_tensor(f"ar_out_{l}_{g}", [2, 129, N // 2], BF,
                              addr_space="Shared")
               for g in range(2)] for l in range(2)]
    hag_in = [[nc.dram_tensor(f"hag_in_{l}_{g}", [D, SH], BF)
               for g in range(2)] for l in range(2)]
    hag_out = [[nc.dram_tensor(f"hag_out_{l}_{g}", [NCORES, D, SH], BF,
                               addr_space="Shared")
                for g in range(2)] for l in range(2)]

    with tile.TileContext(nc) as tc:
        with (
            tc.tile_pool(name="const", bufs=1) as cpool,
            tc.tile_pool(name="xt", bufs=2) as xt_pool,
            tc.tile_pool(name="small", bufs=2) as spool,
            tc.tile_pool(name="kqv", bufs=1) as kqv_pool,
            tc.tile_pool(name="es", bufs=6) as es_pool,
            tc.tile_pool(name="csacc", bufs=2) as cs_pool,
            tc.tile_pool(name="stage", bufs=3) as st_pool,
            tc.tile_pool(name="ybig", bufs=1) as y_pool,
            tc.tile_pool(name="mt", bufs=1) as mt_pool,
            tc.tile_pool(name="z", bufs=5) as z_pool,
            tc.tile_pool(name="ps", bufs=2, space="PSUM") as ps_pool,
            tc.tile_pool(name="ps_p", bufs=3, space="PSUM") as psp_pool,
            tc.tile_pool(name="ps_cs", bufs=1, space="PSUM") as pscs_pool,
        ):
            # ---- load constants ----
            wm = cpool.tile([128, 15 * 128], BF, name="wm_sb")
            nc.scalar.dma_start(
                wm.rearrange("p (i f) -> p i f", i=15),
                wm_in.ap().rearrange("i p f -> p i f"))
            bs = cpool.tile([128, 8], F32, name="bs_sb")
            nc.scalar.dma_start(
                bs.rearrange("p (i f) -> p i f", i=8),
                bs_in.ap().rearrange("i p f -> p i f"))
            ones_m1 = cpool.tile([128, 1], BF, name="ones_m1")
            nc.vector.memset(ones_m1[:], 1.0)
            ones_row = cpool.tile([1, 128], BF, name="ones_row")
            nc.vector.memset(ones_row[:], 1.0)

            def W(l, i):
                base = 7 * l + i if i < 7 else IDENT
                return wm[:, 128 * base:128 * (base + 1)]

            def B(l, i):
                return bs[:, 4 * l + i:4 * l + i + 1]

            ident = wm[:, 128 * IDENT:128 * (IDENT + 1)]

            # ---- load inputs (generation 0) ----
            xgt = []
            for g in range(2):
                t = xt_pool.tile([D, N], BF, name=f"x{g}t_0", tag=f"xt{g}")
                nc.scalar.dma_start(t[:], xgt_in[g][:])
                xgt.append(t)
            xown = []
            for g in range(2):
                t = spool.tile([D, SH], BF, name=f"xown{g}_0", tag=f"xo{g}")
                nc.scalar.dma_start(t[:], xown_in[g])
                xown.append(t)

            hown_final = [None, None]

            for l in range(2):
                # ---- projections ----
                kt = kqv_pool.tile([D, 2 * SH], BF, name=f"kt_{l}", tag="kt",
                                   bufs=2)
                vnat = []
                for g in range(2):
                    ps = ps_pool.tile([128, 512], F32, tag="ps",
                                      name=f"psk_{l}_{g}")
                    nc.tensor.matmul(ps[:], W(l, WK), xown[g][:],
                                     start=True, stop=True)
                    nc.vector.tensor_scalar(kt[:, g * SH:(g + 1) * SH], ps[:],
                                            B(l, BK), None, ADD)
                    # v^T then transpose to natural [t, d] tiles (no bias; bv
                    # is folded in after the softmax division)
                    ps2 = ps_pool.tile([128, 512], F32, tag="ps",
                                       name=f"psv_{l}_{g}")
                    nc.tensor.matmul(ps2[:], W(l, WV), xown[g][:],
                                     start=True, stop=True)
                    vt = st_pool.tile([128, SH], BF, name=f"vt_{l}_{g}",
                                      tag="vt", bufs=2)
                    nc.vector.tensor_copy(vt[:], ps2[:])
                    psv = ps_pool.tile([128, 512], BF, tag="ps",
                                       name=f"psvt_{l}_{g}")
                    for j in range(4):
                        nc.tensor.transpose(psv[:, j * 128:(j + 1) * 128],
                                            vt[:, j * 128:(j + 1) * 128],
                                            ident)
                    vb = st_pool.tile([128, 512], BF, name=f"vn_{l}_{g}",
                                      tag=f"vn{g}", bufs=1)
                    nc.vector.tensor_copy(vb[:], psv[:])
                    vnat += [vb[:, j * 128:(j + 1) * 128] for j in range(4)]
                qt = []
                for g in range(2):
                    q = kqv_pool.tile([D, N], BF, name=f"qt_{l}_{g}",
                                      tag=f"qt{g}", bufs=1)
                    for ic in range(NIC):
                        ps = ps_pool.tile([128, ICW], F32, tag="ps",
                                          name=f"psq_{l}_{g}_{ic}")
                        for h in range(2):
                            nc.tensor.matmul(
                                ps[:, h * 512:(h + 1) * 512], W(l, WQ),
                                xgt[g][:, ic * ICW + h * 512:
                                        ic * ICW + (h + 1) * 512],
                                start=True, stop=True)
                        nc.vector.tensor_scalar(q[:, ic * ICW:(ic + 1) * ICW],
                                                ps[:], B(l, BQ), None, ADD)
                    qt.append(q)

                ls = [None, None]      # local X-only SAGE terms [d, own]
                hown = [None, None]

                def attention(g):
                    for ic in range(NIC):
                        php = [psp_pool.tile([128, 512], F32, tag="ps_p",
                                             name=f"php{h}_{l}_{g}_{ic}")
                               for h in range(2)]
                        csa = cs_pool.tile([128, ICW], BF, tag="cs")
                        for tt in range(NT):
                            ps_s = ps_pool.tile([128, ICW], F32, tag="ps")
                            for h in range(2):
                                nc.tensor.matmul(
                                    ps_s[:, h * 512:(h + 1) * 512],
                                    kt[:, tt * 128:(tt + 1) * 128],
                                    qt[g][:, ic * ICW + h * 512:
                                            ic * ICW + (h + 1) * 512],
                                    start=True, stop=True)
                            es = es_pool.tile([128, ICW], BF, tag="es")
                            nc.scalar.activation(
                                es[:], ps_s[:],
                                mybir.ActivationFunctionType.Exp,
                                scale=INV_SCALE)
                            for h in range(2):
                                nc.tensor.matmul(
                                    php[h][:], vnat[tt],
                                    es[:, h * 512:(h + 1) * 512],
                                    start=(tt == 0), stop=(tt == NT - 1))
                            if tt == 0:
                                nc.vector.tensor_copy(csa[:], es[:])
                            else:
                                nc.vector.tensor_tensor(csa[:], csa[:], es[:],
                                                        ADD)
                        pc = st_pool.tile([128, ICW], BF, tag="pc")
                        cc = st_pool.tile([1, ICW], BF, tag="cc")
                        for h in range(2):
                            nc.vector.tensor_copy(
                                pc[:, h * 512:(h + 1) * 512], php[h][:])
                            ps_c = pscs_pool.tile([1, 512], F32, tag="ps_cs")
                            nc.tensor.matmul(ps_c[:], ones_m1[:],
                                             csa[:, h * 512:(h + 1) * 512],
                                             start=True, stop=True)
                            nc.vector.tensor_copy(
                                cc[:, h * 512:(h + 1) * 512], ps_c[:])
                        hh, icq = divmod(ic, 2)
                        nc.scalar.dma_start(
                            ar_in[l][g][hh, 0:128,
                                        icq * ICW:(icq + 1) * ICW],
                            pc[:])
                        nc.scalar.dma_start(
                            ar_in[l][g][hh, 128:129,
                                        icq * ICW:(icq + 1) * ICW],
                            cc[:])
                        if ic % 2 == 1:
                            nc.gpsimd.collective_compute(
                                "AllReduce", ADD, replica_groups=rg,
                                ins=[ar_in[l][g][hh]],
                                outs=[ar_out[l][g][hh]])

                def xsage(g):
                    """X-only SAGE terms: M @ X @ Wls + X_own @ Wrs (+ bl).
                    Independent of the attention output; fills PE slack."""
                    ybig = y_pool.tile([128, NJ * 128], BF,
                                       name=f"y_{l}_{g}", tag=f"y{g}")
                    for jb in range(NJ // 4):
                        psy = pscs_pool.tile([128, 512], F32, tag="ps_cs",
                                             name=f"psy_{l}_{g}_{jb}")
                        for k in range(4):
                            jt = jb * 4 + k
                            nc.tensor.matmul(psy[:, k * 128:(k + 1) * 128],
                                             xgt[g][:, jt * 128:(jt + 1) * 128],
                                             W(l, WLS), start=True, stop=True)
                        nc.vector.tensor_copy(
                            ybig[:, jb * 512:(jb + 1) * 512], psy[:])
                    ps_a = psp_pool.tile([128, 512], F32, tag="ps_p",
                                         name=f"psa_{l}_{g}")
                    for jb in range(NJ // 4):
                        mtc_t = mt_pool.tile([128, 4 * SH], BF, tag="mtcs",
                                             bufs=4,
                                             name=f"mtc_{l}_{g}_{jb}")
                        nc.scalar.dma_start(
                            mtc_t.rearrange("p (j n) -> p j n", j=4),
                            mtc_in[g].ap()[4 * jb:4 * jb + 4]
                            .rearrange("j p n -> p j n"))
                        for k in range(4):
                            jt = jb * 4 + k
                            nc.tensor.matmul(
                                ps_a[:], ybig[:, jt * 128:(jt + 1) * 128],
                                mtc_t[:, k * SH:(k + 1) * SH],
                                start=(jt == 0), stop=False)
                    nc.tensor.matmul(ps_a[:], W(l, WRS), xown[g][:],
                                     start=False, stop=True)
                    t = spool.tile([128, SH], F32, name=f"ls_{l}_{g}",
                                   tag=f"ls{g}")
                    nc.vector.tensor_scalar(t[:], ps_a[:], B(l, BL), None,
                                            ADD)
                    ls[g] = t

                def tail(g):
                    """Post-AllReduce: softmax epilogue on the full width,
                    local out-part aggregation, h for own rows, AllGather."""
                    pfull = spool.tile([128, N], BF, name=f"pfull_{l}_{g}",
                                       tag="pfull", bufs=1)
                    csrow = spool.tile([1, N], BF, name=f"csrow_{l}_{g}",
                                       tag="csrow")
                    outt = spool.tile([128, N], BF, name=f"outt_{l}_{g}",
                                      tag="outt", bufs=1)
                    for hh in range(2):
                        nc.scalar.dma_start(
                            pfull[:, hh * 2048:(hh + 1) * 2048],
                            ar_out[l][g][hh, 0:128, :])
                        nc.scalar.dma_start(
                            csrow[:, hh * 2048:(hh + 1) * 2048],
                            ar_out[l][g][hh, 128:129, :])
                        for chq in range(4):
                            ch = hh * 4 + chq
                            sl = slice(ch * 512, (ch + 1) * 512)
                            ps_rep = psp_pool.tile([128, 512], F32,
                                                   tag="ps_p",
                                                   name=f"psrep_{l}_{g}_{ch}")
                            nc.tensor.matmul(ps_rep[:], ones_row[:],
                                             csrow[:, sl],
                                             start=True, stop=True)
                            rr = spool.tile([128, 512], F32,
                                            name=f"rr_{l}_{g}", tag="rr")
                            nc.vector.reciprocal_approx_fast(rr[:], ps_rep[:])
                            prod = spool.tile([128, 512], BF,
                                              name=f"prod_{l}_{g}",
                                              tag="prod")
                            nc.vector.tensor_tensor(prod[:], pfull[:, sl],
                                                    rr[:], MULT)
                            nc.vector.tensor_scalar(outt[:, sl], prod[:],
                                                    B(l, BV), None, ADD)
                    # own columns of out (runtime core offset)
                    oown = spool.tile([128, 512], BF, name=f"oown_{l}_{g}",
                                      tag="oown")
                    pid = nc.gpsimd.partition_id()
                    nc.gpsimd.dma_start(oown[:],
                                        outt[:, bass.ds(pid * SH, SH)])
                    # U = out @ Wl1  (natural [j, d] tiles, all 4096 j)
                    ubig = spool.tile([128, N], BF, name=f"ubig_{l}_{g}",
                                      tag="ubig", bufs=1)
                    for jb in range(8):
                        ps_u = pscs_pool.tile([128, 512], F32, tag="ps_cs",
                                              name=f"psu_{l}_{g}_{jb}")
                        for k in range(4):
                            jt = jb * 4 + k
                            nc.tensor.matmul(
                                ps_u[:, k * 128:(k + 1) * 128],
                                outt[:, jt * 128:(jt + 1) * 128],
                                W(l, WL1), start=True, stop=True)
                        nc.vector.tensor_copy(
                            ubig[:, jb * 512:(jb + 1) * 512], ps_u[:])
                    # P2 = M @ U |own + out_own @ Wr1   (to subtract from ls)
                    ps_a2 = psp_pool.tile([128, 512], F32, tag="ps_p",
                                          name=f"psa2_{l}_{g}")
                    for jb in range(NJ // 4):
                        mtc_t = mt_pool.tile([128, 4 * SH], BF, tag="mtcs",
                                             bufs=4,
                                             name=f"mtc2_{l}_{g}_{jb}")
                        nc.scalar.dma_start(
                            mtc_t.rearrange("p (j n) -> p j n", j=4),
                            mtc_in[g].ap()[4 * jb:4 * jb + 4]
                            .rearrange("j p n -> p j n"))
                        for k in range(4):
                            jt = jb * 4 + k
                            nc.tensor.matmul(
                                ps_a2[:], ubig[:, jt * 128:(jt + 1) * 128],
                                mtc_t[:, k * SH:(k + 1) * SH],
                                start=(jt == 0), stop=False)
                    nc.tensor.matmul(ps_a2[:], W(l, WR1N), oown[:],
                                     start=False, stop=True)
                    h = spool.tile([D, SH], BF, name=f"hown_{l}_{g}",
                                   tag=f"xo{g}")
                    if l == 0:
                        t2 = spool.tile([128, 512], F32, name=f"t2_{l}_{g}",
                                        tag="t2")
                        nc.vector.tensor_tensor(t2[:], ls[g][:], ps_a2[:],
                                                SUB)
                        nc.vector.tensor_scalar(h[:], t2[:], 0.0, None, MAX)
                    else:
                        nc.vector.tensor_tensor(h[:], ls[g][:], ps_a2[:],
                                                SUB)
                    hown[g] = h
                    nc.scalar.dma_start(hag_in[l][g][:], h[:])
                    nc.gpsimd.collective_compute(
                        "AllGather", mybir.AluOpType.bypass, replica_groups=rg,
                        ins=[hag_in[l][g][:]], outs=[hag_out[l][g][:]])

                xsage(0)
                xsage(1)
                attention(0)
                tail(0)
                attention(1)
                tail(1)

                # ---- gather new X generation ----
                new_xgt = []
                for g in range(2):
                    t = xt_pool.tile([D, N], BF, name=f"x{g}t_{l + 1}",
                                     tag=f"xt{g}")
                    nc.scalar.dma_start(
                        t.rearrange("p (c n) -> p c n", c=NCORES),
                        hag_out[l][g].ap().rearrange("c p n -> p c n"))
                    new_xgt.append(t)
                xgt = new_xgt
                xown = hown
                if l == 1:
                    hown_final = hown

            # ---- final adjacency: sigmoid(F @ F^T), own 1024 rows ----
            for g in range(2):
                for rt in range(4):
                    lhs = hown_final[g][:, rt * 128:(rt + 1) * 128]
                    for cb in range(8):
                        src = xgt[cb // 4]
                        c0 = (cb % 4) * ICW
                        ps_z = ps_pool.tile([128, ICW], F32, tag="ps")
                        for h in range(2):
                            nc.tensor.matmul(
                                ps_z[:, h * 512:(h + 1) * 512], lhs,
                                src[:, c0 + h * 512:c0 + (h + 1) * 512],
                                start=True, stop=True)
                        z = z_pool.tile([128, ICW], F32, tag="z")
                        nc.scalar.activation(
                            z[:], ps_z[:],
                            mybir.ActivationFunctionType.Sigmoid)
                        nc.scalar.dma_start(
                            out_ext[g, rt * 128:(rt + 1) * 128,
                                    cb * ICW:cb * ICW + 512],
                            z[:, 0:512])
                        nc.gpsimd.dma_start(
                            out_ext[g, rt * 128:(rt + 1) * 128,
                                    cb * ICW + 512:(cb + 1) * ICW],
                            z[:, 512:1024])

    nc.compile()
    return nc


def _host_prep(inputs):
    """Build per-core input maps from the full problem inputs."""
    x1 = np.asarray(inputs["x1"], np.float32)
    x2 = np.asarray(inputs["x2"], np.float32)
    x1t = np.ascontiguousarray(x1.T).astype(BF16)
    x2t = np.ascontiguousarray(x2.T).astype(BF16)

    def norm_adj_t(ei):
        ei = np.asarray(ei)
        A = np.zeros((N, N), np.float32)
        np.add.at(A, (ei[1], ei[0]), 1.0)
        deg = A.sum(1)
        A /= np.maximum(deg, 1.0)[:, None]
        return np.ascontiguousarray(A.T)  # MT[j, n]

    mt = [norm_adj_t(inputs["ei1"]), norm_adj_t(inputs["ei2"])]

    wm = np.zeros((15, 128, 128), np.float32)
    bs = np.zeros((8, 128, 1), np.float32)
    for l, s in enumerate(("1", "2")):
        wm[7 * l + WK] = inputs["Wk" + s]
        wm[7 * l + WQ] = inputs["Wq" + s]
        wm[7 * l + WV] = inputs["Wv" + s]
        wm[7 * l + WLS] = inputs["Wl" + s][:128] + inputs["Wl" + s][128:]
        wm[7 * l + WL1] = inputs["Wl" + s][128:]
        wm[7 * l + WRS] = inputs["Wr" + s][:128] + inputs["Wr" + s][128:]
        wm[7 * l + WR1N] = inputs["Wr" + s][128:]
        bs[4 * l + BK, :, 0] = inputs["bk" + s]
        bs[4 * l + BQ, :, 0] = inputs["bq" + s]
        bs[4 * l + BV, :, 0] = inputs["bv" + s]
        bs[4 * l + BL, :, 0] = inputs["bl" + s]
    wm[IDENT] = np.eye(128)
    wm = wm.astype(BF16)

    in_maps = []
    for c in range(NCORES):
        sl = slice(c * SH, (c + 1) * SH)
        in_maps.append({
            "x1t": x1t,
            "x2t": x2t,
            "xown": np.stack([x1t[:, sl], x2t[:, sl]]),
            "mtc1": np.ascontiguousarray(
                mt[0][:, sl].astype(BF16).reshape(NJ, 128, SH)),
            "mtc2": np.ascontiguousarray(
                mt[1][:, sl].astype(BF16).reshape(NJ, 128, SH)),
            "wm": wm,
            "bs": bs,
        })
    return in_maps


def _assemble(results):
    full = np.empty((2 * N, 2 * N), np.float32)
    for c in range(NCORES):
        o = results[c]["out"]
        full[c * SH:(c + 1) * SH] = o[0]
        full[N + c * SH:N + (c + 1) * SH] = o[1]
    return full


def get_nc():
    if "nc" not in _cache:
        _cache["nc"] = _build_nc()
    return _cache["nc"]


def kernel(**inputs):
    nc = get_nc()
    in_maps = _host_prep(inputs)
    res = run_bass_kernel_spmd(nc, in_maps, core_ids=list(range(NCORES)))
    return _assemble(res.results)
